# revision 31
# baseline (speedup 1.0000x reference)
"""KAN layer (cubic B-spline, uniform grid) for 8 Trainium2 NeuronCores.

Math: with u = 1.5*x + 4.5, basis_j(x) = N(u - j) where N is the uniform
cubic B-spline bump on [0, 4].  N splits into two tent-cubes:
    N(v) = (A^3 - 4*B^3) / 6,  A = relu(2 - |v-2|),  B = relu(1 - |v-2|).
Both tents vanish outside their support, so basis values are exactly zero
out of range and magnitudes stay <= 8 -- safe for fp8e4 with no
cancellation.

Engine split per 512-token group (all four engines balanced):
  Act:    s_j = |1.5*x + (2.5-j)| (Abs->f16); sil = Silu(x) (fp8);
          PSUM drains (Copy x 1/64 -> f16 y).  GpSimd does DMA queueing
          only -- any Pool-engine ALU work trips the power throttler and
          slows every other engine's clock.
  DVE:    bn_j = cbrt4*(1 - s_j) (fast 2x tensor_scalar), then one fused
          custom DVE op per j (KAN_BASIS2_ANT, 8 ALU stages):
            basis_j = sq(2-s)*relu(2-s) - sq(relu(bn))*relu(bn)
                    = A^3 - 4*B^3   -> fp8e4
  PE:     14 fp8 DoubleRow matmuls per 128-token chunk (12 basis-pair +
          2 silu-pair, 2 k-tiles each, 2x rate), one fp32 PSUM bank.

Data parallel over tokens: core c processes batch row c (2048 tokens).
"""

import numpy as np
import ml_dtypes

import concourse.bass as bass
import concourse.mybir as mybir
import concourse.tile as tile
from concourse import bacc
from concourse.bass_utils import run_bass_kernel_spmd

F32 = mybir.dt.float32
F16 = mybir.dt.float16
F8 = mybir.dt.float8e4
ALU = mybir.AluOpType
AF = mybir.ActivationFunctionType
DRM = mybir.MatmulPerfMode.DoubleRow
NP8 = ml_dtypes.float8_e4m3

N_CORES = 8
D = 512          # in_features
O = 512          # out_features
TOK = 2048       # tokens per core
NJ = 6           # spline basis functions
GROUPS = [512, 512, 512, 384, 128]   # token group sizes (sum = TOK)
assert sum(GROUPS) == TOK
DT = D // 128    # d-tiles
NPR = NJ * 2     # basis DoubleRow pairs per chunk
SC = 64.0        # weight scale (fp8 subnormal avoidance)

_prog_cache = {}
last_results = None  # BassKernelResults of the most recent run (for test.py)


def _register_ops():
    """Register the fused basis custom DVE op (idempotent)."""
    import concourse.dve_ops as dve_ops
    from concourse.dve_ops import DveOp
    from concourse.dve_spec import (Spec, Src0, Src1, C0, relu, sq,
                                    lower, _has_src1)
    from concourse.dve_uop import DveOpSpec

    name = "KAN_BASIS2_ANT"
    for op in dve_ops.OPS:
        if op.name == name:
            return op
    a = C0 - Src0
    rb = relu(Src1)
    spec = Spec(
        body=sq(a) * relu(a) - sq(rb) * rb,
        reference=lambda in0, in1, s0, s1, imm2: (
            np.maximum(s0 - in0.astype(np.float32), 0.0) ** 3
            - np.maximum(in1.astype(np.float32), 0.0) ** 3))
    opcode = dve_ops._CUSTOM_DVE_ROW_BASE + len(dve_ops.OPS)
    shas = {}
    for ver in ("v3", "v4"):
        s = DveOpSpec(name=name, opcode=opcode, uops=lower(spec, ver=ver),
                      rd1_en=_has_src1(spec))
        shas[ver] = s.sha(ver)
    op = DveOp(name, spec, subdim=False, uops_sha=shas)
    dve_ops.OPS.append(op)
    dve_ops._SUB_OPCODE_FOR_NAME[name] = opcode
    dve_ops.CUSTOM_DVE_SPECS[name] = spec
    return op


def _build_program():
    basis_op = _register_ops()
    nc = bacc.Bacc("TRN2", target_bir_lowering=False, debug=False,
                   num_devices=N_CORES)
    xT_d = nc.dram_tensor("xT", [D, TOK], F16, kind="ExternalInput").ap()
    w8_d = nc.dram_tensor("w8", [128, NPR, 2, O], F8, kind="ExternalInput").ap()
    sb_d = nc.dram_tensor("s8", [128, 2, 2, O], F8, kind="ExternalInput").ap()
    bc_d = nc.dram_tensor("bcols", [128, 8], F32, kind="ExternalInput").ap()
    y_d = nc.dram_tensor("y", [TOK, O], F16, kind="ExternalOutput").ap()

    with tile.TileContext(nc) as tc:
        with tc.tile_pool(name="const", bufs=1) as cpool, \
             tc.tile_pool(name="xg", bufs=2) as xpool, \
             tc.tile_pool(name="work", bufs=2) as wpool, \
             tc.tile_pool(name="planes", bufs=2) as ppool, \
             tc.tile_pool(name="outp", bufs=2) as opool, \
             tc.tile_pool(name="psum", bufs=7, space="PSUM") as pspool:

            bcols = cpool.tile([128, 8], F32, name="bcols_t", tag="bcols")
            nc.gpsimd.dma_start(bcols[:], bc_d[:])
            w8all = cpool.tile([128, NPR, 2, O], F8, name="w8all", tag="w8all")
            s8all = cpool.tile([128, 2, 2, O], F8, name="s8all", tag="s8all")

            wones = cpool.tile([1, O], F16, name="wones", tag="wones")
            nc.vector.memset(wones[:], 0.0)
            wps = pspool.tile([128, O], F32, name="wps", tag="wps", bufs=1)
            for _w in range(12):
                nc.tensor.matmul(wps[:], wones[:, 0:128], wones[:],
                                 start=True, stop=True)

            GV = 512
            g0 = 0
            for gi, G in enumerate(GROUPS):
                CPG = G // 128
                xg = xpool.tile([128, DT * GV], F16, name="xg", tag="xg")
                xgv = xg[:].rearrange("p (t g) -> p t g", g=GV)[:, :, :G]
                xq = [nc.sync, nc.scalar, nc.sync, nc.scalar]
                for t_ in range(DT):
                    xq[t_].dma_start(
                        xg[:, t_ * GV:t_ * GV + G],
                        xT_d[t_ * 128:(t_ + 1) * 128, g0:g0 + G])
                if gi == 0:
                    nc.sync.dma_start(w8all[:], w8_d[:])
                    nc.scalar.dma_start(s8all[:], sb_d[:])
                basis = []
                sil = ppool.tile([128, DT, GV], F8, name="sil", tag="sil")
                CBRT4 = 1.5874010519682
                for j in range(NJ):
                    s = wpool.tile([128, DT, GV], F16, name=f"s{j}",
                                   tag=f"s{j}")
                    bn = wpool.tile([128, DT, GV], F16, name=f"bn{j}",
                                    tag=f"bn{j}")
                    bj = ppool.tile([128, DT, GV], F8, name=f"b{j}",
                                    tag=f"b{j}")
                    for hd in (slice(0, DT),):
                        nc.scalar.activation(s[:, hd, :G], xgv[:, hd, :],
                                             AF.Abs, bias=bcols[:, j:j + 1],
                                             scale=1.5)
                        nc.vector.tensor_scalar(bn[:, hd, :G], s[:, hd, :G],
                                                -CBRT4, CBRT4,
                                                ALU.mult, ALU.add)
                        nc.vector._custom_dve(basis_op, out=bj[:, hd, :G],
                                              in0=s[:, hd, :G],
                                              in1=bn[:, hd, :G], s0=2.0)
                    basis.append(bj)
                nc.scalar.activation(sil[:, :, :G], xgv, AF.Silu)
                for c in range(CPG):
                    ps = pspool.tile([128, O], F32, name="ps", tag="ps")
                    n_mm = NPR + 2
                    i = 0
                    for j in range(NJ):
                        for tp in range(2):
                            lhsT = basis[j][:, 2 * tp:2 * tp + 2,
                                            c * 128:(c + 1) * 128]
                            nc.tensor.matmul(ps[:], lhsT,
                                             w8all[:, j * 2 + tp, :, :],
                                             start=(i == 0), stop=False,
                                             perf_mode=DRM)
                            i += 1
                    for tp in range(2):
                        lhsT = sil[:, 2 * tp:2 * tp + 2,
                                   c * 128:(c + 1) * 128]
                        nc.tensor.matmul(ps[:], lhsT, s8all[:, tp, :, :],
                                         start=False, stop=(i == n_mm - 1),
                                         perf_mode=DRM)
                        i += 1
                    ot = opool.tile([128, O], F16, name="ot", tag="ot")
                    nc.scalar.activation(ot[:], ps[:], AF.Copy,
                                         bias=0.0, scale=1.0 / SC)
                    nc.gpsimd.dma_start(
                        y_d[g0 + c * 128:g0 + (c + 1) * 128, :], ot[:])
                g0 += G
    nc.compile()
    return nc


def _host_tables(coef, scale_base, scale_sp, bias):
    W = (scale_sp[..., None] * coef).astype(np.float64)        # (O, D, 6)
    w8 = np.empty((128, NPR, 2, O), NP8)
    for j in range(NJ):
        Vj = (SC / 6.0) * W[:, :, j]                           # (O, D)
        for tp in range(2):
            for i in range(2):
                dt_ = 2 * tp + i
                w8[:, j * 2 + tp, i, :] = \
                    Vj[:, dt_ * 128:(dt_ + 1) * 128].T.astype(NP8)
    s8 = np.empty((128, 2, 2, O), NP8)
    sb_scaled = SC * scale_base.astype(np.float64)
    for tp in range(2):
        for i in range(2):
            dt_ = 2 * tp + i
            s8[:, tp, i, :] = \
                sb_scaled[:, dt_ * 128:(dt_ + 1) * 128].T.astype(NP8)
    bcols = np.zeros((128, 8), np.float32)
    for j in range(NJ):
        bcols[:, j] = 2.5 - j
    return (np.ascontiguousarray(w8), np.ascontiguousarray(s8),
            np.ascontiguousarray(bcols))


def kernel(x, coef, scale_base, scale_sp, bias, _trace=False):
    global last_results
    x = np.asarray(x, np.float32)
    coef = np.asarray(coef, np.float32)
    scale_base = np.asarray(scale_base, np.float32)
    scale_sp = np.asarray(scale_sp, np.float32)
    bias = np.asarray(bias, np.float32)
    B, S, Din = x.shape
    assert (B * S, Din) == (N_CORES * TOK, D), (x.shape,)

    if "nc" not in _prog_cache:
        _prog_cache["nc"] = _build_program()
    nc = _prog_cache["nc"]

    w8, s8, bcols = _host_tables(coef, scale_base, scale_sp, bias)
    xflat = x.reshape(N_CORES, TOK, D)
    in_maps = []
    for c in range(N_CORES):
        in_maps.append({
            "xT": np.ascontiguousarray(xflat[c].T.astype(np.float16)),
            "w8": w8, "s8": s8, "bcols": bcols,
        })
    kw = {}
    if _trace:
        kw.update(trace=True)
    last_results = run_bass_kernel_spmd(nc, in_maps,
                                        core_ids=list(range(N_CORES)), **kw)
    y = np.stack([last_results.results[c]["y"] for c in range(N_CORES)], 0)
    y = y.reshape(B, S, O).astype(np.float32)
    if np.any(bias):
        y += bias[None, None, :]
    return y
